# revision 34
# baseline (speedup 1.0000x reference)
"""Multi-head self-attention (B=2, S=2048, D=1024, H=16) on 8 TRN2 NeuronCores.

Sharding: data-parallel over batch (2) x tensor-parallel over head-groups (4).
Core c = b*4 + hg handles batch b, heads hg*4..hg*4+3 (4 heads, 256 features).

Per-core device program (SPMD, identical on all cores):
  - QKV projections for the core's 256 output features (column-parallel)
  - full S x S attention for its 4 heads (softmax without max-subtraction,
    denominators via an appended ones-column in the PV matmul)
  - partial output projection (row-parallel): out_partial^T [1024, 2048]
Host: shards/transposes inputs, sums the 4 partial outputs per batch
(the "all-reduce"), adds bo, and untransposes.

All matmuls run in float32r (TF32-like, ~11 mantissa bits, 1 cycle/row on the
PE vs 4 for plain fp32); accumulation is fp32 in PSUM.
"""

import numpy as np

B, S, D = 2, 2048, 1024
H, DK = 16, 64
NCORES = 8
HG = 4          # head groups (tensor parallel)
HPG = 4         # heads per group
F = HPG * DK    # 256 local features per core
SCALE = 1.0 / np.sqrt(DK)

_compiled = {}


def _build():
    import concourse.bacc as bacc
    import concourse.tile as tile
    from concourse import mybir

    f32 = mybir.dt.float32
    bf16 = mybir.dt.bfloat16
    Exp = mybir.ActivationFunctionType.Exp
    mult = mybir.AluOpType.mult

    nc = bacc.Bacc("TRN2", target_bir_lowering=False, debug=False,
                   enable_asserts=True, num_devices=NCORES)

    xq = nc.dram_tensor("xq", (D, S), bf16, kind="ExternalInput")   # q[b].T
    xk = nc.dram_tensor("xk", (D, S), bf16, kind="ExternalInput")
    xv = nc.dram_tensor("xv", (D, S), bf16, kind="ExternalInput")
    wq = nc.dram_tensor("wq", (D, F), bf16, kind="ExternalInput")   # Wq[rows].T
    wk = nc.dram_tensor("wk", (D, F), bf16, kind="ExternalInput")
    wv = nc.dram_tensor("wv", (D, F), bf16, kind="ExternalInput")
    wo = nc.dram_tensor("wo", (F, D), bf16, kind="ExternalInput")   # Wo[:, cols].T
    bq = nc.dram_tensor("bq", (128, 2), f32, kind="ExternalInput")  # bias, f-tiled
    bk = nc.dram_tensor("bk", (128, 2), f32, kind="ExternalInput")
    bv = nc.dram_tensor("bv", (1, F), f32, kind="ExternalInput")
    out = nc.dram_tensor("out", (D, S), f32, kind="ExternalOutput")  # partial^T

    NDT = D // 128   # 8 d-tiles
    NST = S // 128   # 16 s-tiles (j tiles)
    NSB = S // 512   # 4 s-blocks (i blocks)

    with tile.TileContext(nc) as tc:
        import contextlib
        with contextlib.ExitStack() as ctx:
            consts = ctx.enter_context(tc.tile_pool(name="consts", bufs=1))
            big = ctx.enter_context(tc.tile_pool(name="big", bufs=25))
            atp = ctx.enter_context(tc.tile_pool(name="atp", bufs=9))
            acts = ctx.enter_context(tc.tile_pool(name="acts", bufs=1))
            ostage = ctx.enter_context(tc.tile_pool(name="ostage", bufs=3))
            small = ctx.enter_context(tc.tile_pool(name="small", bufs=2))
            ps = ctx.enter_context(tc.tile_pool(name="ps", bufs=1, space="PSUM"))

            # ---- constants (weights split per d-tile so compute starts early) ----
            wq_sb = consts.tile([128, NDT, F], bf16, tag="wq")
            wk_sb = consts.tile([128, NDT, F], bf16, tag="wk")
            wv_sb = consts.tile([128, NDT, F], bf16, tag="wv")
            for dt in range(NDT):
                nc.sync.dma_start(wq_sb[:, dt, :], wq.ap()[dt * 128:(dt + 1) * 128, :])
            wo_sb = consts.tile([128, 2, D], bf16, tag="wo")
            bq_sb = consts.tile([128, 2], f32, tag="bq")
            bk_sb = consts.tile([128, 2], f32, tag="bk")
            nc.sync.dma_start(bq_sb[:], bq.ap()[:])
            nc.sync.dma_start(bk_sb[:], bk.ap()[:])
            bv_sb = consts.tile([128, F], f32, tag="bv")
            nc.sync.dma_start(bv_sb[:], bv.ap().to_broadcast((128, F)))

            # persistent activations
            # qh/kh: [f, s] transposed projections, per (ft, sb) tiles
            qh_t = [[acts.tile([128, 512], bf16, tag=f"qh{ft}{sb}", name=f"qh{ft}{sb}")
                     for sb in range(NSB)] for ft in range(2)]
            kh_t = [[acts.tile([128, 512], bf16, tag=f"kh{ft}{sb}", name=f"kh{ft}{sb}")
                     for sb in range(NSB)] for ft in range(2)]
            # vh: [s, h, c+1] with ones column at c=64 (PV denominator trick)
            vh_t = [acts.tile([128, HPG, DK + 1], bf16, tag=f"vh{st}", name=f"vh{st}")
                    for st in range(NST)]
            for st in range(NST):
                nc.vector.memset(vh_t[st][:, :, DK:DK + 1], 1.0)
            # y: normalized attention output, [f, s] per (ft, sb)
            y_t = [[acts.tile([128, 512], bf16, tag=f"y{ft}{sb}", name=f"y{ft}{sb}")
                    for sb in range(NSB)] for ft in range(2)]

            # ---- input DMAs (xq, xk, xv streams; tiles stay resident) ----
            from concourse.tile_rust import add_dep_helper

            def alloc_x(pfx):
                return [big.tile([128, S], bf16, tag="big", name=f"{pfx}{dt}")
                        for dt in range(NDT)]

            def load_half(xdram, ts, h0, h1, after=None):
                last = None
                for dt in range(NDT):
                    d = nc.sync.dma_start(ts[dt][:, h0:h1],
                                          xdram.ap()[dt * 128:(dt + 1) * 128, h0:h1])
                    if after is not None and dt == 0:
                        add_dep_helper(d.ins, after.ins, sync=True,
                                       reason="input DMA priority ordering")
                    last = d
                return last

            # ---- projection pass helpers (per (ft, sb) single-bank accum) ----
            def qk_pass(w_sb, b_sb, xts, dst, ft, pfx, gs=(0, 1)):
                for g in gs:
                    accs = [ps.tile([128, 512], f32, tag="w1", bufs=4,
                                    name=f"{pfx}{ft}{g}{j}") for j in range(2)]
                    for dt in range(NDT):
                        for j in range(2):
                            sb = 2 * g + j
                            nc.tensor.matmul(
                                accs[j][:],
                                w_sb[:, dt, ft * 128:(ft + 1) * 128],
                                xts[dt][:, sb * 512:(sb + 1) * 512],
                                start=(dt == 0), stop=(dt == NDT - 1),
                            )
                    for j in range(2):
                        nc.vector.tensor_scalar_add(dst[ft][2 * g + j][:], accs[j][:],
                                                    b_sb[:, ft:ft + 1])

            def v_pass(xvt):
                for g in range(NST // 2):
                    accs = [ps.tile([128, 512], f32, tag="w1", bufs=4,
                                    name=f"vps{g}{j}") for j in range(2)]
                    for dt in range(NDT):
                        for j in range(2):
                            st = 2 * g + j
                            nc.tensor.matmul(
                                accs[j][:, 0:F],
                                xvt[dt][:, st * 128:(st + 1) * 128],
                                wv_sb[:, dt, :],
                                start=(dt == 0), stop=(dt == NDT - 1),
                            )
                    for j in range(2):
                        st = 2 * g + j
                        nc.vector.tensor_tensor(
                            vh_t[st][:, :, 0:DK],
                            accs[j][:, 0:F].rearrange("p (h c) -> p h c", h=HPG),
                            bv_sb[:].rearrange("p (h c) -> p h c", h=HPG),
                            mybir.AluOpType.add,
                        )

            # ---- chunk-level software pipeline: ib N's PV interleaves with
            # ib N+1's scores so the static PE stream never serializes ----
            def scores_chunk(pr, ib, jc):
                ft = pr
                at = atp.tile([128, 4, 512], bf16, tag="at", name=f"at{pr}{ib}{jc}")
                for jj in range(2):
                    jt = jc * 2 + jj
                    sc = ps.tile([128, 2, 512], f32, tag="w2", bufs=2, name="sc")
                    for hh in range(2):
                        base = hh * 64
                        nc.tensor.matmul(
                            sc[:, hh, :],
                            kh_t[ft][jt // 4][base:base + 64,
                                              (jt % 4) * 128:(jt % 4 + 1) * 128],
                            qh_t[ft][ib][base:base + 64, :],
                            start=True, stop=True,
                            tile_position=(base, 0),
                        )
                    nc.scalar.activation(
                        at[:, jj * 2:jj * 2 + 2, :],
                        sc[:, :, :],
                        Exp, scale=float(SCALE),
                    )
                return at

            def pv_chunk(pr, pv_ps, at, jc):
                for hh in range(2):
                    h = 2 * pr + hh
                    for jj in range(2):
                        jt = 2 * jc + jj
                        nc.tensor.matmul(
                            pv_ps[hh][0:DK + 1, :],
                            vh_t[jt][:, h, :],
                            at[:, 2 * jj + hh, :],
                            start=(jt == 0), stop=(jt == NST - 1),
                        )

            def finish_ib(pr, ib, pv_ps, with_outproj=False):
                ft = pr
                for hh in range(2):
                    den = small.tile([1, 512], f32, tag="den")
                    nc.vector.tensor_copy(den[:], pv_ps[hh][DK:DK + 1, :])
                    rec = small.tile([1, 512], f32, tag="rec")
                    nc.vector.reciprocal_approx_fast(rec[:], den[:])
                    rb = small.tile([64, 512], f32, tag="rb")
                    nc.gpsimd.partition_broadcast(rb[:], rec[:])
                    nc.vector.tensor_tensor(
                        y_t[ft][ib][hh * 64:hh * 64 + 64, :],
                        pv_ps[hh][0:DK, :],
                        rb[:],
                        mult,
                    )
                if with_outproj:
                    outproj_sb(ib)

            # ---- output projection for one s-block ----
            def outproj_sb(sb):
                for et in range(NDT):
                    po = ps.tile([128, 512], f32, tag="w1", bufs=4,
                                 name=f"po{et}{sb}")
                    for ft in range(2):
                        nc.tensor.matmul(
                            po[:],
                            wo_sb[:, ft, et * 128:(et + 1) * 128],
                            y_t[ft][sb][:],
                            start=(ft == 0), stop=(ft == 1),
                        )
                    o_sb = ostage.tile([128, 512], f32, tag="ost", name=f"os{et}{sb}")
                    nc.vector.tensor_copy(o_sb[:], po[:])
                    nc.sync.dma_start(
                        out.ap()[et * 128:(et + 1) * 128, sb * 512:(sb + 1) * 512],
                        o_sb[:],
                    )

            # ---- phase schedule: start pair-0 attention early; ft=1
            # projections fill PE gaps during the ACT-paced attention ----
            xkt = alloc_x("xk")
            xqt = alloc_x("xq")
            xvt = alloc_x("xv")
            for dt in range(NDT):
                nc.sync.dma_start(wk_sb[:, dt, :], wk.ap()[dt * 128:(dt + 1) * 128, :])
            lk = load_half(xk, xkt, 0, S)                      # keys first: QK needs all of them
            lqA = load_half(xq, xqt, 0, S // 2, after=lk)      # queries for ib 0-1
            for dt in range(NDT):
                nc.sync.dma_start(wv_sb[:, dt, :], wv.ap()[dt * 128:(dt + 1) * 128, :])
            lv = load_half(xv, xvt, 0, S, after=lqA)           # values before first PV
            lqB = load_half(xq, xqt, S // 2, S, after=lv)      # queries for ib 2-3
            for ft in range(2):
                nc.sync.dma_start(wo_sb[:, ft, :], wo.ap()[ft * 128:(ft + 1) * 128, :])
            qk_pass(wk_sb, bk_sb, xkt, kh_t, 0, "psk")
            qk_pass(wq_sb, bq_sb, xqt, qh_t, 0, "psq", gs=(0,))

            seq = [(0, 0), (0, 1), (0, 2), (0, 3), (1, 0), (1, 1), (1, 2), (1, 3)]
            prev = None  # (pr, ib, pv_ps, at_list)
            for pr, ib in seq:
                at_list = []
                ppv = None
                for jc in range(NST // 2):
                    at_list.append(scores_chunk(pr, ib, jc))
                    if prev is not None:
                        if jc == 0:
                            ppv = [ps.tile([128, 512], f32, tag="w1", bufs=4,
                                           name=f"pv{prev[0]}{prev[1]}_{i}")
                                   for i in range(2)]
                        pv_chunk(prev[0], ppv, prev[3][jc], jc)
                if prev is not None:
                    finish_ib(prev[0], prev[1], ppv, with_outproj=(prev[0] == 1))
                if (pr, ib) == (0, 0):
                    v_pass(xvt)
                elif (pr, ib) == (0, 1):
                    qk_pass(wq_sb, bq_sb, xqt, qh_t, 0, "psq", gs=(1,))
                    qk_pass(wk_sb, bk_sb, xkt, kh_t, 1, "psk")
                elif (pr, ib) == (0, 3):
                    qk_pass(wq_sb, bq_sb, xqt, qh_t, 1, "psq")
                prev = (pr, ib, None, at_list)

            # drain the last ib's PV un-interleaved
            lpv = [ps.tile([128, 512], f32, tag="w1", bufs=4, name=f"pvlast{i}")
                   for i in range(2)]
            for jc in range(NST // 2):
                pv_chunk(prev[0], lpv, prev[3][jc], jc)
            finish_ib(prev[0], prev[1], lpv, with_outproj=True)

    nc.compile()
    return nc


def _get_nc():
    if "nc" not in _compiled:
        _compiled["nc"] = _build()
    return _compiled["nc"]


def kernel(q, k, v, Wq, bq, Wk, bk, Wv, bv, Wo, bo):
    outp, _ = _run(q, k, v, Wq, bq, Wk, bk, Wv, bv, Wo, bo)
    return outp


def _run(q, k, v, Wq, bq, Wk, bk, Wv, bv, Wo, bo, **run_kwargs):
    from concourse.bass_utils import run_bass_kernel_spmd

    nc = _get_nc()

    q = np.asarray(q, np.float32)
    k = np.asarray(k, np.float32)
    v = np.asarray(v, np.float32)
    Wq = np.asarray(Wq, np.float32)
    Wk = np.asarray(Wk, np.float32)
    Wv = np.asarray(Wv, np.float32)
    Wo = np.asarray(Wo, np.float32)
    bq = np.asarray(bq, np.float32)
    bk = np.asarray(bk, np.float32)
    bv = np.asarray(bv, np.float32)
    bo = np.asarray(bo, np.float32)

    import ml_dtypes
    bf = ml_dtypes.bfloat16
    xqT = [np.ascontiguousarray(q[b].T).astype(bf) for b in range(B)]
    xkT = [np.ascontiguousarray(k[b].T).astype(bf) for b in range(B)]
    xvT = [np.ascontiguousarray(v[b].T).astype(bf) for b in range(B)]

    in_maps = []
    for c in range(NCORES):
        b, hg = divmod(c, HG)
        rows = slice(hg * F, (hg + 1) * F)
        in_maps.append({
            "xq": xqT[b], "xk": xkT[b], "xv": xvT[b],
            "wq": np.ascontiguousarray(Wq[rows].T).astype(bf),
            "wk": np.ascontiguousarray(Wk[rows].T).astype(bf),
            "wv": np.ascontiguousarray(Wv[rows].T).astype(bf),
            "wo": np.ascontiguousarray(Wo[:, rows].T).astype(bf),
            "bq": np.ascontiguousarray(bq[rows].reshape(2, 128).T),
            "bk": np.ascontiguousarray(bk[rows].reshape(2, 128).T),
            "bv": np.ascontiguousarray(bv[rows].reshape(1, F)),
        })

    res = run_bass_kernel_spmd(nc, in_maps, core_ids=list(range(NCORES)), **run_kwargs)

    outp = np.empty((B, S, D), np.float32)
    for b in range(B):
        acc = res.results[b * HG]["out"].astype(np.float32)
        for hg in range(1, HG):
            acc = acc + res.results[b * HG + hg]["out"]
        outp[b] = acc.T + bo[None, :]
    return outp, res
